# revision 1
# baseline (speedup 1.0000x reference)
"""DeformableBiomarkerAttention Trainium2 kernel.

Strategy: pure data-parallel over batch (8 batches per NeuronCore, 8 cores).
Per core:
  - trilinear sampling of 32 points x 8 batches from the 8x8x8 feature grid
    via indirect-DMA gathers (2x2x2 corners; x-adjacent rows fetched in pairs)
  - sample/in/out projections as PE matmuls with activations held in
    transposed (channel-on-partition) layout; weights pre-transposed on host
  - single-query MHA (12 heads) with head-masked q for scores, free-dim
    softmax, PE head-broadcast of attention weights
  - output = broadcast(attn_out * confidence) written as contiguous DMAs
"""

import numpy as np

import concourse.bass as bass
import concourse.mybir as mybir
import concourse.tile as tile
from concourse import bass_utils
from concourse.tile_rust import add_dep_helper

F32 = mybir.dt.float32
I32 = mybir.dt.int32
ALU = mybir.AluOpType
ACTF = mybir.ActivationFunctionType

E = 768
CH = 6            # number of 128-channel chunks
NB = 32           # points per batch
BPC = 8           # batches per core
FULLN = 513
NCORES = 8
B = 64
ROWS = BPC * NB   # 256 sampled rows per core
NG = 2            # partition groups of 128 rows
NH = 12           # heads
HD = 64           # head dim


def _body(ctx, tc):
    nc = tc.nc

    def inp(name, shape, dt=F32):
        return nc.dram_tensor(name, shape, dt, kind="ExternalInput").ap()

    # ---- DRAM I/O (per-core shard; host prepares these layouts) ----
    x = inp("x", [BPC * FULLN, E])            # flattened x shard
    bioT = inp("bioT", [128, CH, BPC])        # bio_embed^T chunked
    base = inp("base", [NG, 128, 3])          # base_coords tiled over batches
    offs = inp("offs", [NG, 128, 3])          # offsets
    conf = inp("conf", [BPC, 1])
    rowbase = inp("rowbase", [NG, 128, 1])    # (local_batch*513 + 1) per row
    mul3 = inp("mul3", [128, 3])              # (1, 8, 64)
    hselC = inp("hselC", [128, CH, NH])       # head-select mask per chunk
    bsel = inp("bsel", [NH, CH * 128])        # head-row -> channel broadcast
    onehots = inp("onehots", [BPC, BPC * 128])  # batch-row -> 128-row bcast
    identity = inp("identity", [128, 128])
    wst = inp("wst", [128, CH, E])            # sample_proj_w^T chunked
    wqt = inp("wqt", [128, CH, E])
    wkt = inp("wkt", [128, CH, E])
    wvt = inp("wvt", [128, CH, E])
    wot = inp("wot", [128, CH, E])
    bs = inp("bs", [128, CH])                 # biases, channel-on-partition
    bq = inp("bq", [128, CH])                 # pre-scaled by 1/8
    bk = inp("bk", [128, CH])
    bv = inp("bv", [128, CH])
    bo_bc = inp("bo_bc", [BPC, E])            # out bias broadcast over batch
    out = nc.dram_tensor("out", [BPC * FULLN, E], F32, kind="ExternalOutput").ap()

    cpool = ctx.enter_context(tc.tile_pool(name="consts", bufs=1))
    wpool = ctx.enter_context(tc.tile_pool(name="weights", bufs=1))
    gpool = ctx.enter_context(tc.tile_pool(name="gather", bufs=3))
    tpool = ctx.enter_context(tc.tile_pool(name="tmp", bufs=2))
    bcpool = ctx.enter_context(tc.tile_pool(name="bcast", bufs=3))
    spool = ctx.enter_context(tc.tile_pool(name="small", bufs=1))
    pp = ctx.enter_context(tc.tile_pool(name="ps", bufs=6, space="PSUM"))

    _psn = [0]

    def psum(shape):
        _psn[0] += 1
        return pp.tile(shape, F32, tag="ps", name=f"ps{_psn[0]}")


    # ---- weight / const loads (sync HWDGE queue, issued first) ----
    w_tiles = {}
    w_dmas = {}
    for name, ap in (("wqt", wqt), ("wst", wst), ("wkt", wkt), ("wvt", wvt),
                     ("wot", wot)):
        t = wpool.tile([128, CH, E], F32, tag=name)
        w_dmas[name] = nc.sync.dma_start(out=t[:], in_=ap[:])
        w_tiles[name] = t

    c_dmas = {}

    def load_const(name, ap, shape):
        t = cpool.tile(shape, F32, tag=name)
        c_dmas[name] = nc.sync.dma_start(out=t[:], in_=ap[:])
        return t

    bioT_t = load_const("bioT", bioT, [128, CH, BPC])
    base_g, offs_g, rowb_g = [], [], []
    for g in range(NG):
        bt = cpool.tile([128, 3], F32, tag=f"base{g}", name=f"base{g}")
        nc.sync.dma_start(out=bt[:], in_=base[g])
        base_g.append(bt)
        ot = cpool.tile([128, 3], F32, tag=f"offs{g}", name=f"offs{g}")
        nc.sync.dma_start(out=ot[:], in_=offs[g])
        offs_g.append(ot)
        rt = cpool.tile([128, 1], F32, tag=f"rowb{g}", name=f"rowb{g}")
        nc.sync.dma_start(out=rt[:], in_=rowbase[g])
        rowb_g.append(rt)
    conf_t = load_const("conf", conf, [BPC, 1])
    mul3_t = load_const("mul3", mul3, [128, 3])
    hsel_t = load_const("hselC", hselC, [128, CH, NH])
    bsel_t = load_const("bsel", bsel, [NH, CH * 128])
    oneh_t = load_const("onehots", onehots, [BPC, BPC * 128])
    iden_t = load_const("identity", identity, [128, 128])
    bs_t = load_const("bs", bs, [128, CH])
    bq_t = load_const("bq", bq, [128, CH])
    bk_t = load_const("bk", bk, [128, CH])
    bv_t = load_const("bv", bv, [128, CH])
    bo_t = load_const("bo_bc", bo_bc, [BPC, E])

    # ---- coords -> corner indices + trilinear weights (DVE) ----
    # coords order is (x, y, z); flat grid index = 64*z + 8*y + x.
    samp_nat = []   # per group: [128, 768] sampled (rows on partitions)
    wsum_last = []
    for g in range(NG):
        c_t = spool.tile([128, 3], F32, tag=f"c{g}", name=f"c{g}")
        nc.vector.tensor_add(out=c_t[:], in0=base_g[g][:], in1=offs_g[g][:])
        nc.vector.tensor_scalar(out=c_t[:], in0=c_t[:], scalar1=1.0,
                                scalar2=-1.0, op0=ALU.min, op1=ALU.max)
        i_t = spool.tile([128, 3], F32, tag=f"i{g}", name=f"i{g}")
        nc.vector.tensor_scalar(out=i_t[:], in0=c_t[:], scalar1=1.0,
                                scalar2=3.5, op0=ALU.add, op1=ALU.mult)
        # floor(i) robust to the f32->int rounding mode: r = round(i);
        # i0 = r - (i < r)
        ri_t = spool.tile([128, 3], I32, tag=f"ri{g}", name=f"ri{g}")
        nc.vector.tensor_copy(out=ri_t[:], in_=i_t[:])
        rf_t = spool.tile([128, 3], F32, tag=f"rf{g}", name=f"rf{g}")
        nc.vector.tensor_copy(out=rf_t[:], in_=ri_t[:])
        neg_t = spool.tile([128, 3], F32, tag=f"neg{g}", name=f"neg{g}")
        nc.vector.tensor_tensor(out=neg_t[:], in0=i_t[:], in1=rf_t[:],
                                op=ALU.is_lt)
        i0_t = spool.tile([128, 3], F32, tag=f"i0{g}", name=f"i0{g}")
        nc.vector.tensor_sub(out=i0_t[:], in0=rf_t[:], in1=neg_t[:])
        nc.vector.tensor_scalar(out=i0_t[:], in0=i0_t[:], scalar1=6.0,
                                scalar2=None, op0=ALU.min)
        w_t = spool.tile([128, 3], F32, tag=f"w{g}", name=f"w{g}")
        nc.vector.tensor_sub(out=w_t[:], in0=i_t[:], in1=i0_t[:])
        omw_t = spool.tile([128, 3], F32, tag=f"omw{g}", name=f"omw{g}")
        nc.vector.tensor_scalar(out=omw_t[:], in0=w_t[:], scalar1=-1.0,
                                scalar2=1.0, op0=ALU.mult, op1=ALU.add)
        pr_t = spool.tile([128, 3], F32, tag=f"pr{g}", name=f"pr{g}")
        nc.vector.tensor_mul(out=pr_t[:], in0=i0_t[:], in1=mul3_t[:])
        ib_t = spool.tile([128, 1], F32, tag=f"ib{g}", name=f"ib{g}")
        nc.vector.reduce_sum(out=ib_t[:], in_=pr_t[:], axis=mybir.AxisListType.X)
        nc.vector.tensor_add(out=ib_t[:], in0=ib_t[:], in1=rowb_g[g][:])

        # pair index per (cz, cy): row of (z0+cz, y0+cy, x0); x0/x0+1 fetched
        # together as one contiguous 2-row read.
        idxf_t = spool.tile([128, 4], F32, tag=f"idxf{g}", name=f"idxf{g}")
        wc_t = spool.tile([128, 8], F32, tag=f"wc{g}", name=f"wc{g}")
        wyz_t = spool.tile([128, 4], F32, tag=f"wyz{g}", name=f"wyz{g}")
        for j, (cz, cy) in enumerate(((0, 0), (0, 1), (1, 0), (1, 1))):
            nc.vector.tensor_scalar(out=idxf_t[:, j:j + 1], in0=ib_t[:],
                                    scalar1=float(64 * cz + 8 * cy),
                                    scalar2=None, op0=ALU.add)
            ysel = w_t[:, 1:2] if cy else omw_t[:, 1:2]
            zsel = w_t[:, 2:3] if cz else omw_t[:, 2:3]
            nc.vector.tensor_mul(out=wyz_t[:, j:j + 1], in0=ysel, in1=zsel)
            nc.vector.tensor_mul(out=wc_t[:, 2 * j:2 * j + 1],
                                 in0=wyz_t[:, j:j + 1], in1=omw_t[:, 0:1])
            nc.vector.tensor_mul(out=wc_t[:, 2 * j + 1:2 * j + 2],
                                 in0=wyz_t[:, j:j + 1], in1=w_t[:, 0:1])
        idx8f_t = spool.tile([128, 8], F32, tag=f"idx8f{g}", name=f"idx8f{g}")
        for j in range(4):
            for xb in range(2):
                nc.vector.tensor_scalar(
                    out=idx8f_t[:, 2 * j + xb:2 * j + xb + 1],
                    in0=idxf_t[:, j:j + 1], scalar1=float(xb),
                    scalar2=None, op0=ALU.add)
        idx_t = spool.tile([128, 8], I32, tag=f"idx{g}", name=f"idx{g}")
        nc.vector.tensor_copy(out=idx_t[:], in_=idx8f_t[:])

        # ---- gathers + incremental weighted sum ----
        acc = cpool.tile([128, E], F32, tag=f"samp{g}", name=f"samp{g}")
        for c8 in range(8):
            corner = gpool.tile([128, E], F32, tag="corner", name="corner")
            nc.gpsimd.indirect_dma_start(
                out=corner[:], out_offset=None, in_=x[:],
                in_offset=bass.IndirectOffsetOnAxis(ap=idx_t[:, c8:c8 + 1],
                                                    axis=0),
            )
            if c8 == 0:
                nc.vector.tensor_scalar(
                    out=acc[:], in0=corner[:],
                    scalar1=wc_t[:, c8:c8 + 1], scalar2=None, op0=ALU.mult)
            else:
                tmp = tpool.tile([128, E], F32, tag="wtmp", name="wtmp")
                nc.vector.tensor_scalar(
                    out=tmp[:], in0=corner[:],
                    scalar1=wc_t[:, c8:c8 + 1], scalar2=None, op0=ALU.mult)
                last_op = nc.vector.tensor_add(out=acc[:], in0=acc[:],
                                               in1=tmp[:])
        samp_nat.append(acc)
        wsum_last.append(last_op)

    # ---- q projection: qT[co] = (Wq @ bio^T) * (1/8) + bq/8 ----
    qT = []
    for co in range(CH):
        ps = psum([128, BPC])
        for ci in range(CH):
            nc.tensor.matmul(
                out=ps[:], lhsT=w_tiles["wqt"][:, ci, 128 * co:128 * (co + 1)],
                rhs=bioT_t[:, ci, :], start=(ci == 0), stop=(ci == CH - 1))
        qt = cpool.tile([128, BPC], F32, tag=f"qT{co}", name=f"qT{co}")
        nc.scalar.activation(out=qt[:], in_=ps[:], func=ACTF.Identity,
                             bias=bq_t[:, co:co + 1], scale=0.125)
        qT.append(qt)

    # ---- transpose sampled -> sampT (channel-on-partition) ----
    sampT = []
    for ch in range(CH):
        st = cpool.tile([128, ROWS], F32, tag=f"sampT{ch}", name=f"sampT{ch}")
        sampT.append(st)
    for g in range(NG):
        for ch in range(CH):
            ps = psum([128, 128])
            nc.tensor.transpose(
                out=ps[:], in_=samp_nat[g][:, 128 * ch:128 * (ch + 1)],
                identity=iden_t[:])
            nc.scalar.copy(out=sampT[ch][:, 128 * g:128 * (g + 1)], in_=ps[:])

    # ---- sample / K / V projections (transposed activations) ----
    def proj_pass(wname, rhs_tiles, bias_t, out_tag):
        outs = []
        for co in range(CH):
            ps = psum([128, ROWS])
            for ci in range(CH):
                nc.tensor.matmul(
                    out=ps[:],
                    lhsT=w_tiles[wname][:, ci, 128 * co:128 * (co + 1)],
                    rhs=rhs_tiles[ci][:],
                    start=(ci == 0), stop=(ci == CH - 1))
            o = cpool.tile([128, ROWS], F32, tag=f"{out_tag}{co}", name=f"{out_tag}{co}")
            nc.scalar.activation(out=o[:], in_=ps[:], func=ACTF.Identity,
                                 bias=bias_t[:, co:co + 1], scale=1.0)
            outs.append(o)
        return outs

    sampPT = proj_pass("wst", sampT, bs_t, "sampPT")
    kT = proj_pass("wkt", sampPT, bk_t, "kT")
    vT = proj_pass("wvt", sampPT, bv_t, "vT")

    # ---- scores: [12 heads, 8 batches, 32 points] ----
    qexp = []
    qexp_ops = []
    for ch in range(CH):
        qe = cpool.tile([128, BPC, NH], F32, tag=f"qexp{ch}", name=f"qexp{ch}")
        qexp_ops.append(nc.vector.tensor_mul(
            out=qe[:],
            in0=qT[ch][:].unsqueeze(2).to_broadcast([128, BPC, NH]),
            in1=hsel_t[:, ch, :].unsqueeze(1).to_broadcast([128, BPC, NH])))
        qexp.append(qe)
    sc_ps = psum([NH, BPC, NB])
    for b in range(BPC):
        for ci in range(CH):
            nc.tensor.matmul(
                out=sc_ps[:, b, :], lhsT=qexp[ci][:, b, :],
                rhs=kT[ci][:, NB * b:NB * (b + 1)],
                start=(ci == 0), stop=(ci == CH - 1))

    # ---- softmax over points ----
    m_t = spool.tile([NH, BPC, 1], F32, tag="mx", name="mx")
    nc.vector.reduce_max(out=m_t[:], in_=sc_ps[:], axis=mybir.AxisListType.X)
    es_t = spool.tile([NH, BPC, NB], F32, tag="esub", name="esub")
    nc.vector.tensor_sub(out=es_t[:], in0=sc_ps[:],
                         in1=m_t[:].to_broadcast([NH, BPC, NB]))
    ex_t = spool.tile([NH, BPC, NB], F32, tag="ex", name="ex")
    nc.scalar.activation(out=ex_t[:], in_=es_t[:], func=ACTF.Exp)
    s_t = spool.tile([NH, BPC, 1], F32, tag="sm", name="sm")
    nc.vector.reduce_sum(out=s_t[:], in_=ex_t[:], axis=mybir.AxisListType.X)
    r_t = spool.tile([NH, BPC, 1], F32, tag="rc", name="rc")
    nc.vector.reciprocal(out=r_t[:], in_=s_t[:])
    at_t = spool.tile([NH, BPC, NB], F32, tag="attn", name="attn")
    attn_op = nc.vector.tensor_mul(out=at_t[:], in0=ex_t[:],
                                   in1=r_t[:].to_broadcast([NH, BPC, NB]))

    # ---- broadcast attn rows to channel layout; ctx reduction ----
    ctxT = cpool.tile([128, CH, BPC], F32, tag="ctxT", name="ctxT")
    ctx_ops = []
    for ch in range(CH):
        ps = psum([128, BPC * NB])
        nc.tensor.matmul(
            out=ps[:], lhsT=bsel_t[:, 128 * ch:128 * (ch + 1)],
            rhs=at_t[:], start=True, stop=True)
        abc = tpool.tile([128, BPC, NB], F32, tag="abc", name="abc")
        nc.scalar.copy(out=abc[:], in_=ps[:])
        prod = tpool.tile([128, BPC, NB], F32, tag="prod", name="prod")
        nc.vector.tensor_mul(
            out=prod[:],
            in0=vT[ch][:].rearrange("p (b n) -> p b n", n=NB),
            in1=abc[:])
        ctx_ops.append(nc.vector.reduce_sum(out=ctxT[:, ch, :].unsqueeze(2),
                                            in_=prod[:],
                                            axis=mybir.AxisListType.X))

    # ---- out projection + bias + confidence ----
    outfin = cpool.tile([BPC, E], F32, tag="outfin", name="outfin")
    for half in range(2):
        sl = slice(384 * half, 384 * (half + 1))
        ps = psum([BPC, 384])
        for ci in range(CH):
            nc.tensor.matmul(
                out=ps[:], lhsT=ctxT[:, ci, :],
                rhs=w_tiles["wot"][:, ci, sl],
                start=(ci == 0), stop=(ci == CH - 1))
        nc.vector.tensor_add(out=outfin[:, sl], in0=ps[:], in1=bo_t[:][:, sl])
    outfin_op = nc.vector.tensor_scalar(out=outfin[:], in0=outfin[:],
                                        scalar1=conf_t[:][:, 0:1],
                                        scalar2=None, op0=ALU.mult)

    # ---- broadcast each batch row to 128 partitions and store ----
    for b in range(BPC):
        bt = bcpool.tile([128, E], F32, tag="bt", name="bt")
        for half in range(2):
            sl = slice(384 * half, 384 * (half + 1))
            ps = psum([128, 384])
            nc.tensor.matmul(
                out=ps[:], lhsT=oneh_t[:][:, 128 * b:128 * (b + 1)],
                rhs=outfin[:, sl], start=True, stop=True)
            nc.scalar.copy(out=bt[:, sl], in_=ps[:])
        for tchunk in range(4):
            r0 = FULLN * b + 128 * tchunk
            nc.sync.dma_start(out=out[r0:r0 + 128, :], in_=bt[:])
        nc.sync.dma_start(out=out[FULLN * b + 512:FULLN * b + 513, :],
                          in_=bt[0:1, :])


_NO_SPLIT_TYPES = {"InstUnconditionalBranch", "InstConditionalBranch"}


def _split_waits(nc, max_waits=1):
    # walrus (CoreV3) accepts only one sync-wait command per compute
    # instruction; move extra waits onto injected same-engine NoOps placed
    # immediately before the instruction (semantics unchanged).
    import bass_rust
    k = 0
    for fn in nc.m.functions:
        for bb in fn.blocks:
            insts = bb.instructions
            i = 0
            while i < len(insts):
                inst = insts[i]
                si = inst.sync_info
                if (type(inst).__name__ not in _NO_SPLIT_TYPES
                        and si is not None
                        and si.on_wait and len(si.on_wait) > max_waits):
                    waits = list(si.on_wait)
                    extra, keep = waits[:-max_waits], waits[-max_waits:]
                    for w in extra:
                        k += 1
                        nop = bass_rust.InstNoOp(name=f"I-wsplit-{k}",
                                                 engine=inst.engine,
                                                 ins=[], outs=[])
                        nop.sync_info = bass_rust.SyncInfo(on_wait=[w],
                                                           on_update=[])
                        insts.insert(i, nop)
                        i += 1
                    inst.sync_info = bass_rust.SyncInfo(
                        on_wait=keep, on_update=list(si.on_update or []))
                i += 1
    return k


def build(split=True):
    from contextlib import ExitStack

    nc = bass.Bass("TRN2", debug=False, num_devices=NCORES)
    with tile.TileContext(nc) as tc, ExitStack() as es:
        _body(es, tc)
    if split:
        # needed for the walrus compile; CoreSim can't replay injected nops
        _split_waits(nc)
    return nc


def host_prep(inputs):
    """Build per-core in_maps from full inputs (layout marshalling only)."""
    x = np.ascontiguousarray(inputs["x"], dtype=np.float32)
    bio = np.ascontiguousarray(inputs["bio_embed"], dtype=np.float32)
    base = np.ascontiguousarray(inputs["base_coords"], dtype=np.float32)
    offsets = np.ascontiguousarray(inputs["offsets"], dtype=np.float32)
    confidence = np.ascontiguousarray(inputs["confidence"], dtype=np.float32)
    wsp = np.asarray(inputs["sample_proj_w"], dtype=np.float32)
    bsp = np.asarray(inputs["sample_proj_b"], dtype=np.float32)
    win = np.asarray(inputs["in_proj_w"], dtype=np.float32)
    bin_ = np.asarray(inputs["in_proj_b"], dtype=np.float32)
    wout = np.asarray(inputs["out_proj_w"], dtype=np.float32)
    bout = np.asarray(inputs["out_proj_b"], dtype=np.float32)

    def chunkT(w):  # [E, E] -> [128, CH, E] of w^T
        return np.ascontiguousarray(
            w.T.reshape(CH, 128, E).transpose(1, 0, 2))

    def chunkb(v):  # [E] -> [128, CH]
        return np.ascontiguousarray(v.reshape(CH, 128).T)

    consts = {
        "wst": chunkT(wsp),
        "wqt": chunkT(win[:E]),
        "wkt": chunkT(win[E:2 * E]),
        "wvt": chunkT(win[2 * E:]),
        "wot": chunkT(wout),
        "bs": chunkb(bsp),
        "bq": chunkb(bin_[:E] * 0.125),
        "bk": chunkb(bin_[E:2 * E]),
        "bv": chunkb(bin_[2 * E:]),
        "mul3": np.tile(np.array([1.0, 8.0, 64.0], np.float32), (128, 1)),
        "rowbase": ((np.arange(ROWS) // NB) * FULLN + 1.0).astype(
            np.float32).reshape(NG, 128, 1),
        "identity": np.eye(128, dtype=np.float32),
        "base": np.tile(base, (BPC, 1)).reshape(NG, 128, 3),
    }
    hsel = np.zeros((128, CH, NH), np.float32)
    for ch in range(CH):
        for p in range(128):
            hsel[p, ch, (ch * 128 + p) // HD] = 1.0
    consts["hselC"] = hsel
    bsel = np.zeros((NH, CH * 128), np.float32)
    for ch in range(CH):
        for j in range(128):
            bsel[(ch * 128 + j) // HD, ch * 128 + j] = 1.0
    consts["bsel"] = bsel
    oneh = np.zeros((BPC, BPC * 128), np.float32)
    for b in range(BPC):
        oneh[b, 128 * b:128 * (b + 1)] = 1.0
    consts["onehots"] = oneh
    consts["bo_bc"] = np.tile(bout[None, :], (BPC, 1))

    in_maps = []
    for c in range(NCORES):
        bsl = slice(BPC * c, BPC * (c + 1))
        bio_c = bio[bsl]  # [8, 768]
        m = dict(consts)
        m["x"] = x[bsl].reshape(BPC * FULLN, E)
        m["bioT"] = np.ascontiguousarray(
            bio_c.T.reshape(CH, 128, BPC).transpose(1, 0, 2))
        m["offs"] = offsets[bsl].reshape(NG, 128, 3)
        m["conf"] = confidence[bsl].reshape(BPC, 1)
        in_maps.append(m)
    return in_maps


_NC = None


def kernel(**inputs):
    global _NC
    if _NC is None:
        _NC = build()
    in_maps = host_prep(inputs)
    res = bass_utils.run_bass_kernel_spmd(_NC, in_maps,
                                          core_ids=list(range(NCORES)))
    outs = [res.results[c]["out"].reshape(BPC, FULLN, E)
            for c in range(NCORES)]
    return np.concatenate(outs, axis=0)

